# revision 1
# baseline (speedup 1.0000x reference)
"""Trainium2 Bass kernel for nn_DFTParallelRecon: polar-grid DFT CT reconstruction.

Self-contained: builds static geometry tables on host, compiles two SPMD Bass
programs (8 NeuronCores each; core = (BC, band, frame)), runs them sequentially,
and combines partial outputs on host.
"""
import sys
sys.path.insert(0, '/opt/trn_rl_repo')
import numpy as np
import concourse.bass as bass
import concourse.mybir as mybir
from concourse import bacc
from concourse.tile import TileContext
from concourse.masks import make_identity
from concourse.bass_utils import run_bass_kernel_spmd

"""Final tables: core = (BC b, band r, frame f). Two classes: bands {0,3}, {1,2}.
Each class -> one SPMD program over 8 cores (b2 x r2 x f2).
"""
import numpy as np

FM = 2048; V = 512; FN = 2048; M = 512; N_DET = 512; A_DET = 1.0; PIX = 0.5
TWO_PI = 2.0 * np.pi
C0 = 512; NB = 1024; BAND = 256
CHUNK = 128; NCH = 2
QSEG = 32; NG = CHUNK // QSEG
JSEG = 256; NJSEG = NB // JSEG
PIECE_CX = 255
DST_CAP16 = 2046


def polar_pix():
    """All nonzero pixels with reference-exact f32 bilinear data."""
    v = ((np.arange(FM, dtype=np.float32) - np.float32((FM - 1) / 2))
         * np.float32(1.0 / (FM * PIX))).astype(np.float32)
    ky, kx = np.meshgrid(v, v, indexing='xy')
    mk = np.abs(kx + 1j * ky).astype(np.float32)
    th = np.arctan2(ky, kx).astype(np.float32)
    theta = ((th + np.float32(TWO_PI)) / np.float32(TWO_PI) * np.float32(V)).astype(np.float32)
    k = (mk * np.float32(FN * A_DET) + np.float32((FN - 1) / 2)).astype(np.float32)
    theta_norm = ((theta - np.float32(V)) / np.float32(V)).astype(np.float32)
    k_norm = ((k - np.float32((FN - 1) / 2)) / np.float32(FN // 2)).astype(np.float32)
    ix = ((k_norm + np.float32(1.0)) * np.float32(0.5) * np.float32(FN - 1)).astype(np.float32)
    iy = ((theta_norm + np.float32(1.0)) * np.float32(0.5) * np.float32(2 * V - 1)).astype(np.float32)
    x0 = np.floor(ix).astype(np.int64); y0 = np.floor(iy).astype(np.int64)
    wx1 = (ix - x0).astype(np.float32); wy1 = (iy - y0).astype(np.float32)
    I, J = np.meshgrid(np.arange(FM), np.arange(FM), indexing='ij')
    c = (FM - 1) / 2.0
    u = I - c; w = J - c
    inA = np.abs(u) >= np.abs(w)
    v0 = (x0 >= 0) & (x0 <= FN - 1)
    v1 = (x0 + 1 >= 0) & (x0 + 1 <= FN - 1)
    sel = v0 | v1
    ii, jj = np.nonzero(sel)
    P = {}
    P['i'] = ii; P['j'] = jj
    P['frame'] = (~inA[ii, jj]).astype(np.int8)
    P['Y'] = y0[ii, jj]; P['x0'] = x0[ii, jj]
    _wx1 = wx1[ii, jj]; _wy1 = wy1[ii, jj]
    f0 = v0[ii, jj].astype(np.float32); f1 = v1[ii, jj].astype(np.float32)
    P['w0a'] = (1 - _wx1) * (1 - _wy1) * f0
    P['w1a'] = _wx1 * (1 - _wy1) * f1
    P['w0b'] = (1 - _wx1) * _wy1 * f0
    P['w1b'] = _wx1 * _wy1 * f1
    return P


def core_pix(P, r, f, mirror):
    """Pixel arrays for core (band r, frame f): local coords + chunk/slot/rank.
    mirror=True relabels fi -> BAND-1-fi, fj -> NB-1-fj so that the mirror band's
    table ranges align with its partner's; G tables absorb the flip."""
    frow = np.where(P['frame'] == 0, P['i'], P['j'])
    fcol = np.where(P['frame'] == 0, P['j'], P['i'])
    sel = (P['frame'] == f) & (frow >= C0 + r * BAND) & (frow < C0 + (r + 1) * BAND)
    d = {}
    for k in ('Y', 'x0', 'w0a', 'w1a', 'w0b', 'w1b'):
        d[k] = P[k][sel]
    d['fi'] = (frow[sel] - C0 - r * BAND).astype(np.int64)
    d['fj'] = (fcol[sel] - C0).astype(np.int64)
    d['mirror'] = mirror
    if mirror:
        d['fi'] = (BAND - 1) - d['fi']
        d['fj'] = (NB - 1) - d['fj']
    n = len(d['Y'])
    Ys = np.unique(d['Y'])
    assert len(Ys) <= NCH * CHUNK, f"{len(Ys)} wedges"
    ypos = {y: i for i, y in enumerate(Ys)}
    g = np.array([ypos[y] for y in d['Y']], np.int64)
    d['ch'] = g // CHUNK
    d['part'] = g % CHUNK
    d['Ylist'] = np.concatenate([Ys, np.full(NCH * CHUNK - len(Ys), Ys[-1])]).reshape(NCH, CHUNK)
    d['n'] = n
    d['q'] = d['fi'] % CHUNK
    d['C'] = d['fi'] // CHUNK
    return d


def runlen(key_sorted):
    n = len(key_sorted)
    same = np.concatenate([[False], key_sorted[1:] == key_sorted[:-1]])
    run = np.zeros(n, np.int64)
    for i in range(1, n):
        run[i] = run[i - 1] + 1 if same[i] else 0
    return run


def assign_kc(d, xlo):
    """k index within (ch, part, s) cell, and m within (ch, part, C, q)."""
    s = d['x0'] - xlo
    order = np.lexsort((d['fj'], d['fi'], s, d['part'], d['ch']))
    cell = (d['ch'][order] * CHUNK + d['part'][order]) * 4096 + s[order]
    kk = np.zeros(d['n'], np.int64); kk[order] = runlen(cell)
    order2 = np.lexsort((d['fj'], d['q'], d['C'], d['part'], d['ch']))
    cell2 = ((d['ch'][order2] * CHUNK + d['part'][order2]) * 2 + d['C'][order2]) * CHUNK + d['q'][order2]
    mm = np.zeros(d['n'], np.int64); mm[order2] = runlen(cell2)
    d['s'] = s; d['k'] = kk; d['m'] = mm


class ClassMeta:
    pass


def build_class(P, bands):
    """Build meta + per-core tables for one class. Cores: (b, r, f) for r in bands."""
    cores = []          # pixel dicts, one per (r, f); b doesn't affect tables
    for r in bands:
        for f in (0, 1):
            cores.append((r, f, core_pix(P, r, f, mirror=(r >= 2))))
    S = ClassMeta()
    S.bands = bands
    S.xlo = min(int(d['x0'].min()) for _, _, d in cores)
    xext = max(int(d['x0'].max()) for _, _, d in cores) - S.xlo + 1
    for _, _, d in cores:
        assign_kc(d, S.xlo)
    # pieces: smallest NP with per-window rank count <= PIECE_CX
    for NP in range(4, 24):
        pw = int(np.ceil(xext / NP))
        ok = True
        for _, _, d in cores:
            cnt = np.zeros((NCH, CHUNK, NP), np.int64)
            np.add.at(cnt, (d['ch'], d['part'], np.minimum(d['s'] // pw, NP - 1)), 1)
            if cnt.max() > PIECE_CX:
                ok = False; break
        if ok:
            break
    assert ok
    S.NP = NP; S.pw = pw; S.xpad = pw * NP
    # per (ch, piece) K maxed over cores
    Kmap = np.ones((NCH, NP), np.int64)
    for _, _, d in cores:
        cnt = np.zeros((NCH, CHUNK, S.xpad), np.int64)
        np.add.at(cnt, (d['ch'], d['part'], d['s']), 1)
        for w in range(NCH):
            for pc in range(NP):
                Kmap[w, pc] = max(Kmap[w, pc], int(cnt[w, :, pc * pw:(pc + 1) * pw].max()))
    S.pieces = [[(pc * pw, (pc + 1) * pw, int(Kmap[w, pc])) for pc in range(NP)]
                for w in range(NCH)]
    S.wt_off = []; S.sc1_off = []
    for w in range(NCH):
        wo = []; so = []; cw = 0; cs = 0
        for (a, b, K) in S.pieces[w]:
            wo.append(cw); cw += (b - a) * 4 * K
            so.append(cs); cs += (b - a) * K * 4
        S.wt_off.append(wo); S.sc1_off.append(so)
    S.wt_w = max(sum((b - a) * 4 * K for (a, b, K) in S.pieces[w]) for w in range(NCH))
    S.ncpx = PIECE_CX * NP
    # compact positions (per core)
    for _, _, d in cores:
        piece = d['s'] // pw
        order = np.lexsort((d['k'], d['s'], d['part'], piece, d['ch']))
        key = (d['ch'][order] * NP + piece[order]) * CHUNK + d['part'][order]
        run = runlen(key)
        cpos = np.zeros(d['n'], np.int64); cpos[order] = run
        assert cpos.max() < PIECE_CX
        d['piece'] = piece
        d['cpos'] = cpos + piece * PIECE_CX
    # outbox tiles per (w, C)
    S.Tn = {}
    for w in range(NCH):
        for C in (0, 1):
            t = 1
            for _, _, d in cores:
                msk = (d['ch'] == w) & (d['C'] == C)
                if msk.any():
                    t = max(t, int(d['m'][msk].max()) + 1)
            S.Tn[(w, C)] = t
    S.inbox_off = {}; S.inbox_w = {}
    for C in (0, 1):
        off = []; cur = 0
        for w in range(NCH):
            off.append(cur); cur += CHUNK * S.Tn[(w, C)] * 2
        S.inbox_off[C] = off; S.inbox_w[C] = cur
    # sc2 ranges per (w, C, g) maxed over cores
    S.sc2_rng = {}
    for _, _, d in cores:
        for w in range(NCH):
            for C in (0, 1):
                for g in range(NG):
                    msk = (d['ch'] == w) & (d['C'] == C) & (d['q'] // QSEG == g)
                    if not msk.any():
                        continue
                    lo, hi = int(d['cpos'][msk].min()), int(d['cpos'][msk].max()) + 1
                    key = (w, C, g)
                    if key in S.sc2_rng:
                        lo = min(lo, S.sc2_rng[key][0]); hi = max(hi, S.sc2_rng[key][1])
                    S.sc2_rng[key] = (lo, hi)
    S.sc2_off = {}; cur = 0
    for w in range(NCH):
        for C in (0, 1):
            for g in range(NG):
                lo, hi = S.sc2_rng.get((w, C, g), (0, 1))
                S.sc2_rng[(w, C, g)] = (lo, hi)
                S.sc2_off[(w, C, g)] = cur
                cur += (hi - lo) * 4
    S.sc2_w = cur
    # sc3 ranges per (C, jg)
    S.sc3_rng = {}
    for _, _, d in cores:
        for C in (0, 1):
            offs = S.inbox_off[C]
            ipos = offs_ipos(d, S, C)
            for jg in range(NJSEG):
                msk = (d['C'] == C) & (d['fj'] // JSEG == jg)
                if not msk.any():
                    continue
                lo, hi = int(ipos[msk].min()), int(ipos[msk].max()) + 2
                key = (C, jg)
                if key in S.sc3_rng:
                    lo = min(lo, S.sc3_rng[key][0]); hi = max(hi, S.sc3_rng[key][1])
                S.sc3_rng[key] = (lo, hi)
    S.sc3_off = {}; cur = 0
    for C in (0, 1):
        for jg in range(NJSEG):
            lo, hi = S.sc3_rng.get((C, jg), (0, 1))
            S.sc3_rng[(C, jg)] = (lo, hi)
            S.sc3_off[(C, jg)] = cur
            cur += (hi - lo) * 2
    S.sc3_w = cur
    # stage-3 q-chunk trim
    qlo = 8; qhi = 0
    for _, _, d in cores:
        qlo = min(qlo, int(d['fj'].min()) // CHUNK)
        qhi = max(qhi, int(d['fj'].max()) // CHUNK + 1)
    S.qlo, S.qhi = qlo, qhi
    # emit per-core tables
    tabs = {}
    for r, f, d in cores:
        tabs[(r, f)] = emit_core(d, S, r)
    return S, tabs


def offs_ipos(d, S, C):
    offs = S.inbox_off[C]
    ipos = np.zeros(d['n'], np.int64)
    for w in range(NCH):
        mw = d['ch'] == w
        ipos[mw] = offs[w] + (d['part'][mw] * S.Tn[(w, C)] + d['m'][mw]) * 2
    return ipos


def stage1_C():
    dx = A_DET; dk = 1.0 / (FN * dx)
    x0 = -(N_DET - 1) / 2 * dx; k0 = -(FN - 1) / 2 * dk
    m32 = np.arange(N_DET, dtype=np.float32)
    n32 = np.arange(FN, dtype=np.float32)
    ph_pre = (np.float32(TWO_PI * (k0 * dx)) * m32).astype(np.float32)
    pre = np.exp(-1j * ph_pre.astype(np.float64))
    inner = (np.float32(dk) * n32 + np.float32(k0)).astype(np.float32)
    ph_post = (np.float32(TWO_PI * x0) * inner).astype(np.float32)
    post = dx * np.exp(-1j * ph_post.astype(np.float64))
    mm = np.arange(N_DET, dtype=np.float64)
    nn = np.arange(FN, dtype=np.float64)
    Wm = np.exp(-1j * TWO_PI * np.outer(mm, nn) / FN)
    return ((pre[:, None] * Wm) * post[None, :]).astype(np.complex64)


def stage3_G():
    dx = PIX; dk = 1.0 / (FM * dx)
    x0 = -(FM - 1) / 2 * dx; k0 = -(FM - 1) / 2 * dk
    ar32 = np.arange(FM, dtype=np.float32)
    ph_pre = (np.float32(TWO_PI * (x0 * dk)) * ar32).astype(np.float32)
    pre = np.exp(1j * ph_pre.astype(np.float64))
    inner = (np.float32(dx) * ar32 + np.float32(x0)).astype(np.float32)
    ph_post = (np.float32(TWO_PI * k0) * inner).astype(np.float32)
    post = np.exp(1j * ph_post.astype(np.float64))
    lo = (FM - M) // 2
    p = np.arange(lo, lo + M)
    mm = np.arange(C0, C0 + NB)
    G = (dk * post[p])[:, None] * np.exp(1j * TWO_PI * np.outer(p, mm) / FM) * pre[mm][None, :]
    return G.astype(np.complex64)


_CM = None; _G = None


def emit_core(d, S, r):
    global _CM, _G
    if _CM is None:
        _CM = stage1_C(); _G = stage3_G()
    mirror = d['mirror']
    t = {}
    t['viewA'] = (d['Ylist'] % V).astype(np.int32)
    t['viewB'] = ((d['Ylist'] + 1) % V).astype(np.int32)
    n = d['n']
    ch, part, s, k, piece, cpos = d['ch'], d['part'], d['s'], d['k'], d['piece'], d['cpos']
    wt = np.zeros((NCH, CHUNK, S.wt_w), np.float16)
    sc1 = np.full((NCH, CHUNK, S.wt_w), -1, np.int16)
    for w in range(NCH):
        for pc, (a, b, K) in enumerate(S.pieces[w]):
            msk = (ch == w) & (piece == pc)
            if not msk.any():
                continue
            off = S.wt_off[w][pc]
            for di, nm in enumerate(('w0a', 'w1a', 'w0b', 'w1b')):
                pos = off + ((s[msk] - a) * 4 + di) * K + k[msk]
                wt[w, part[msk], pos] = d[nm][msk].astype(np.float16)
            soff = S.sc1_off[w][pc]
            cpl = cpos[msk] - pc * PIECE_CX
            for h in range(4):
                src = soff + ((s[msk] - a) * K + k[msk]) * 4 + h
                sc1[w, part[msk], src] = (cpl * 4 + h).astype(np.int16)
    t['wt'] = wt; t['sc1'] = sc1
    sc2 = np.full((CHUNK, S.sc2_w), -1, np.int16)
    for w in range(NCH):
        for C in (0, 1):
            Tn = S.Tn[(w, C)]
            for g in range(NG):
                lo, hi = S.sc2_rng[(w, C, g)]
                off = S.sc2_off[(w, C, g)]
                msk = (ch == w) & (d['C'] == C) & (d['q'] // QSEG == g)
                if not msk.any():
                    continue
                ql = d['q'][msk] - g * QSEG
                dstp = (ql * Tn + d['m'][msk]) * 2
                srcp = (cpos[msk] - lo) * 4
                assert int(dstp.max()) * 2 + 3 < QSEG * Tn * 4 <= DST_CAP16
                for h in range(4):
                    sc2[part[msk], off + srcp + h] = (dstp * 2 + h).astype(np.int16)
    t['sc2'] = sc2
    sc3 = np.full((CHUNK, S.sc3_w), -1, np.int16)
    for C in (0, 1):
        ipos = offs_ipos(d, S, C)
        for jg in range(NJSEG):
            lo, hi = S.sc3_rng[(C, jg)]
            off = S.sc3_off[(C, jg)]
            msk = (d['C'] == C) & (d['fj'] // JSEG == jg)
            if not msk.any():
                continue
            jl = d['fj'][msk] - jg * JSEG
            for h in range(4):
                ri = h // 2; half = h % 2
                sc3[d['q'][msk], off + (ipos[msk] - lo + ri) * 2 + half] = (
                    (jl * 2 + ri) * 2 + half).astype(np.int16)
    t['sc3'] = sc3
    # stage-1 C window [512, xpad+1, 2]
    xwin = np.zeros((N_DET, S.xpad + 1), np.complex64)
    hi = min(S.xlo + S.xpad + 1, FN)
    xwin[:, :hi - S.xlo] = _CM[:, S.xlo:hi]
    t['cmat'] = np.ascontiguousarray(np.stack([xwin.real, xwin.imag], -1).astype(np.float32))
    fi_loc = np.arange(BAND)
    if mirror:
        fi_loc = (BAND - 1) - fi_loc
    rows = r * BAND + fi_loc
    Gr = _G[:, rows].T
    t['grA'] = np.ascontiguousarray(np.stack([Gr.real, Gr.imag], -1).astype(np.float32))
    t['grB'] = np.ascontiguousarray(np.stack([-Gr.imag, Gr.real], -1).astype(np.float32))
    fj_loc = np.arange(S.qlo * CHUNK, S.qhi * CHUNK)
    if mirror:
        fj_loc = (NB - 1) - fj_loc
    Gq = _G.T[fj_loc]
    t['gcA'] = np.ascontiguousarray(Gq.real.astype(np.float32))
    t['gcB'] = np.ascontiguousarray((-Gq.imag).astype(np.float32))
    return t


def build_all():
    P = polar_pix()
    SA, tabsA = build_class(P, (0, 3))
    SB, tabsB = build_class(P, (1, 2))
    return (SA, tabsA), (SB, tabsB)


# ---------------- mock (device semantics) ----------------

def local_scatter_np(dst16, data16, idx16):
    dst16[:] = 0
    for prt in range(dst16.shape[0]):
        ii = idx16[prt]
        msk = ii >= 0
        dst16[prt, ii[msk].astype(np.int64)] = data16[prt, np.nonzero(msk)[0]]


def mock_core(sgm_b, t, S):
    Cm = t['cmat'][..., 0] + 1j * t['cmat'][..., 1]
    inbox = {C: np.zeros((CHUNK, S.inbox_w[C]), np.float32) for C in (0, 1)}
    compacts = []
    for w in range(NCH):
        Pa = sgm_b[t['viewA'][w]].astype(np.complex64) @ Cm
        Pb = sgm_b[t['viewB'][w]].astype(np.complex64) @ Cm
        Pa = np.stack([Pa.real, Pa.imag], -1).astype(np.float32)
        Pb = np.stack([Pb.real, Pb.imag], -1).astype(np.float32)
        compact = np.zeros((CHUNK, S.ncpx * 4), np.int16)
        for pc, (a, b, K) in enumerate(S.pieces[w]):
            pw = b - a
            wtp = t['wt'][w][:, S.wt_off[w][pc]:S.wt_off[w][pc] + pw * 4 * K]
            wtp = wtp.reshape(CHUNK, pw, 4, K).astype(np.float32)
            Vp = (wtp[:, :, 0, :, None] * Pa[:, a:b, None, :]
                  + wtp[:, :, 1, :, None] * Pa[:, a + 1:b + 1, None, :]
                  + wtp[:, :, 2, :, None] * Pb[:, a:b, None, :]
                  + wtp[:, :, 3, :, None] * Pb[:, a + 1:b + 1, None, :]).astype(np.float32)
            V16 = np.ascontiguousarray(Vp).view(np.int16).reshape(CHUNK, -1)
            idx = t['sc1'][w][:, S.sc1_off[w][pc]:S.sc1_off[w][pc] + pw * K * 4]
            local_scatter_np(compact[:, pc * PIECE_CX * 4:(pc + 1) * PIECE_CX * 4], V16, idx)
        compacts.append(compact)
    for C in (0, 1):
        for w in range(NCH):
            Tn = S.Tn[(w, C)]
            outbox = np.zeros((CHUNK, CHUNK, Tn, 2), np.float32)
            ob16 = outbox.view(np.int16).reshape(CHUNK, -1)
            for g in range(NG):
                lo, hi = S.sc2_rng[(w, C, g)]
                off = S.sc2_off[(w, C, g)]
                idx = t['sc2'][:, off:off + (hi - lo) * 4]
                local_scatter_np(ob16[:, g * QSEG * Tn * 4:(g + 1) * QSEG * Tn * 4],
                                 compacts[w][:, lo * 4:hi * 4], idx)
            ib = inbox[C]; ioff = S.inbox_off[C][w]
            for m in range(Tn):
                for ri in range(2):
                    ib[:, ioff + m * 2 + ri: ioff + CHUNK * Tn * 2: Tn * 2] = \
                        outbox[:, :, m, ri].T
    fkbuf = {}
    for C in (0, 1):
        fk = np.zeros((CHUNK, NB, 2), np.float32)
        fk16 = fk.view(np.int16).reshape(CHUNK, -1)
        ib16 = inbox[C].view(np.int16).reshape(CHUNK, -1)
        for jg in range(NJSEG):
            lo, hi = S.sc3_rng[(C, jg)]
            off = S.sc3_off[(C, jg)]
            idx = t['sc3'][:, off:off + (hi - lo) * 2]
            local_scatter_np(fk16[:, jg * JSEG * 4:(jg + 1) * JSEG * 4],
                             ib16[:, lo * 2:hi * 2], idx)
        fkbuf[C] = fk
    # stage 3 (single frame)
    fkc = np.concatenate([fkbuf[0], fkbuf[1]], axis=0)      # [256p, NB, 2]
    nq = (S.qhi - S.qlo) * CHUNK
    fkq = fkc[:, S.qlo * CHUNK:S.qhi * CHUNK]
    Tq = np.zeros((nq, 512, 2), np.float32)
    for ri in range(2):
        Tq[..., ri] = (np.einsum('pq,pa->qa', fkq[..., 0], t['grA'][..., ri])
                       + np.einsum('pq,pa->qa', fkq[..., 1], t['grB'][..., ri]))
    U = t['gcA'].T @ Tq[..., 0] + t['gcB'].T @ Tq[..., 1]
    return U.astype(np.float32)   # out[b-idx, a-idx] = U_f[a, b]


def full_mock(sgm, classes):
    rec = np.zeros((2, M, M), np.float32)
    for (S, tabs) in classes:
        for (r, f), t in tabs.items():
            for b in range(2):
                out = mock_core(sgm[b, 0], t, S)
                rec[b] += out.T if f == 0 else out
    return rec.reshape(2, 1, M, M)






F32 = mybir.dt.float32
F32R = mybir.dt.float32r
F16 = mybir.dt.float16
I16 = mybir.dt.int16


def build_program(S, use_f32r=False, repeat=1):
    nc = bacc.Bacc("TRN2", target_bir_lowering=False)
    xw = S.xpad + 1
    nq = S.qhi - S.qlo

    def mmcast(ap):
        return ap.bitcast(F32R) if use_f32r else ap

    sgmTa = nc.dram_tensor("sgmTa", [NCH, N_DET, CHUNK], F32, kind="ExternalInput")
    sgmTb = nc.dram_tensor("sgmTb", [NCH, N_DET, CHUNK], F32, kind="ExternalInput")
    cmat = nc.dram_tensor("cmat", [N_DET, xw, 2], F32, kind="ExternalInput")
    wt = nc.dram_tensor("wt", [NCH, CHUNK, S.wt_w], F16, kind="ExternalInput")
    sc1 = nc.dram_tensor("sc1", [NCH, CHUNK, S.wt_w], I16, kind="ExternalInput")
    sc2 = nc.dram_tensor("sc2", [CHUNK, S.sc2_w], I16, kind="ExternalInput")
    sc3 = nc.dram_tensor("sc3", [CHUNK, S.sc3_w], I16, kind="ExternalInput")
    grA = nc.dram_tensor("grA", [2 * CHUNK, 512, 2], F32, kind="ExternalInput")
    grB = nc.dram_tensor("grB", [2 * CHUNK, 512, 2], F32, kind="ExternalInput")
    gcA = nc.dram_tensor("gcA", [nq * CHUNK, 512], F32, kind="ExternalInput")
    gcB = nc.dram_tensor("gcB", [nq * CHUNK, 512], F32, kind="ExternalInput")
    out = nc.dram_tensor("out", [512, 512], F32, kind="ExternalOutput")

    with TileContext(nc) as tc:
      for _rep in range(repeat):
        with (
            tc.tile_pool(name="const", bufs=1) as constp,
            tc.tile_pool(name="compact", bufs=1) as cp,
            tc.tile_pool(name="fkpool", bufs=1) as fkp,
            tc.tile_pool(name="psum", bufs=2, space="PSUM") as psp,
            tc.tile_pool(name="psum2", bufs=2, space="PSUM") as psp2,
        ):
            ident = constp.tile([CHUNK, CHUNK], F32)
            make_identity(nc, ident[:])
            cmt = []
            for kc in range(4):
                t = constp.tile([CHUNK, xw, 2], F32, tag=f"cm{kc}")
                nc.sync.dma_start(out=t[:], in_=cmat[kc * CHUNK:(kc + 1) * CHUNK])
                cmt.append(t)
            grt = {}
            for nm, dram in (("A", grA), ("B", grB)):
                for kc in range(2):
                    t = constp.tile([CHUNK, 512, 2], F32, tag=f"gr{nm}{kc}")
                    nc.sync.dma_start(out=t[:], in_=dram[kc * CHUNK:(kc + 1) * CHUNK])
                    grt[(nm, kc)] = t

            compacts = []
            for w in range(NCH):
                cpt = cp.tile([CHUNK, S.ncpx * 4], I16, tag=f"cpt{w}", name=f"cpt{w}")
                compacts.append(cpt)

            phase1 = tc.tile_pool(name="phase1", bufs=2)
            pp = vp = phase1.__enter__()
            for w in range(NCH):
                sg = {}
                for nm, dram in (("a", sgmTa), ("b", sgmTb)):
                    for kc in range(4):
                        t = pp.tile([CHUNK, CHUNK], F32, tag=f"sg{nm}{kc}")
                        nc.sync.dma_start(out=t[:], in_=dram[w, kc * CHUNK:(kc + 1) * CHUNK])
                        sg[(nm, kc)] = t
                Pab = {}
                for nm in ("a", "b"):
                    P = pp.tile([CHUNK, xw, 2], F32, tag=f"P{nm}")
                    Pab[nm] = P
                    for ri in range(2):
                        xs = 0
                        while xs < xw:
                            pl = min(512, xw - xs)
                            ps = psp.tile([CHUNK, 512], F32, tag="s1")
                            for kc in range(4):
                                nc.tensor.matmul(
                                    ps[:, :pl],
                                    mmcast(sg[(nm, kc)][:]),
                                    mmcast(cmt[kc][:, xs:xs + pl, ri]),
                                    start=(kc == 0), stop=(kc == 3))
                            nc.any.tensor_copy(P[:, xs:xs + pl, ri], ps[:, :pl])
                            xs += pl
                # V pieces + sc1
                for pc, (a, b, K) in enumerate(S.pieces[w]):
                    pw = b - a
                    wtile = vp.tile([CHUNK, pw, 4, K], F16, tag="wt")
                    nc.sync.dma_start(
                        out=wtile[:],
                        in_=wt[w, :, S.wt_off[w][pc]:S.wt_off[w][pc] + pw * 4 * K])
                    idxt = vp.tile([CHUNK, pw * K * 4], I16, tag="sc1i")
                    nc.sync.dma_start(
                        out=idxt[:],
                        in_=sc1[w, :, S.sc1_off[w][pc]:S.sc1_off[w][pc] + pw * K * 4])
                    V = vp.tile([CHUNK, pw, K, 2], F32, tag="V")
                    t1 = vp.tile([CHUNK, pw, K, 2], F32, tag="t1")
                    shp = (CHUNK, pw, K, 2)
                    terms = [("a", 0, 0), ("a", 1, 1), ("b", 0, 2), ("b", 1, 3)]
                    first = True
                    for nm, dx, di in terms:
                        wap = wtile[:, :, di, :, None].to_broadcast(shp)
                        pap = Pab[nm][:, a + dx:b + dx, None, :].to_broadcast(shp)
                        dst = V if first else t1
                        nc.vector.tensor_tensor(out=dst[:], in0=wap, in1=pap,
                                                op=mybir.AluOpType.mult)
                        if not first:
                            nc.vector.tensor_tensor(out=V[:], in0=V[:], in1=t1[:],
                                                    op=mybir.AluOpType.add)
                        first = False
                    vflat = V[:].rearrange("p a k r -> p (a k r)").bitcast(I16)
                    nc.gpsimd.local_scatter(
                        compacts[w][:, pc * PIECE_CX * 4:(pc + 1) * PIECE_CX * 4],
                        vflat, idxt[:],
                        channels=CHUNK, num_elems=PIECE_CX * 4, num_idxs=pw * K * 4)

            phase1.__exit__(None, None, None)
            # routing + fk
            phase2 = tc.tile_pool(name="phase2", bufs=2)
            rp = phase2.__enter__()
            fkt = {}
            for C in (0, 1):
                inbox = rp.tile([CHUNK, S.inbox_w[C]], F32, tag=f"inbox{C}", bufs=1)
                for w in range(NCH):
                    Tn = S.Tn[(w, C)]
                    ob = rp.tile([CHUNK, CHUNK, Tn, 2], F32, tag="outbox")
                    for g in range(NG):
                        lo, hi = S.sc2_rng[(w, C, g)]
                        off = S.sc2_off[(w, C, g)]
                        idxt = rp.tile([CHUNK, (hi - lo) * 4], I16, tag="sc2i")
                        nc.sync.dma_start(out=idxt[:], in_=sc2[:, off:off + (hi - lo) * 4])
                        dst = ob[:, g * QSEG:(g + 1) * QSEG, :, :]
                        dst = dst.rearrange("p q t r -> p (q t r)").bitcast(I16)
                        nc.gpsimd.local_scatter(
                            dst, compacts[w][:, lo * 4:hi * 4], idxt[:],
                            channels=CHUNK, num_elems=QSEG * Tn * 4,
                            num_idxs=(hi - lo) * 4)
                    ioff = S.inbox_off[C][w]
                    isec = inbox[:, ioff:ioff + CHUNK * Tn * 2]
                    isec = isec.rearrange("p (l t r) -> p l t r", t=Tn, r=2)
                    for m in range(Tn):
                        for ri in range(2):
                            pt = psp.tile([CHUNK, CHUNK], F32, tag="tp")
                            nc.tensor.transpose(pt[:], ob[:, :, m, ri], ident[:])
                            nc.any.tensor_copy(isec[:, :, m, ri], pt[:])
                fk = fkp.tile([CHUNK, NB, 2], F32, tag=f"fk{C}")
                fkt[C] = fk
                ibflat = inbox[:].rearrange("p x -> p x").bitcast(I16)
                for jg in range(NJSEG):
                    lo, hi = S.sc3_rng[(C, jg)]
                    off = S.sc3_off[(C, jg)]
                    idxt = rp.tile([CHUNK, (hi - lo) * 2], I16, tag="sc3i")
                    nc.sync.dma_start(out=idxt[:], in_=sc3[:, off:off + (hi - lo) * 2])
                    dst = fk[:, jg * JSEG:(jg + 1) * JSEG, :]
                    dst = dst.rearrange("p j r -> p (j r)").bitcast(I16)
                    nc.gpsimd.local_scatter(
                        dst, ibflat[:, lo * 2:hi * 2], idxt[:],
                        channels=CHUNK, num_elems=JSEG * 4, num_idxs=(hi - lo) * 2)

            phase2.__exit__(None, None, None)
            # stage 3
            phase3 = tc.tile_pool(name="phase3", bufs=2)
            s3p = phase3.__enter__()
            Tt = []
            for qc in range(nq):
                qs = (S.qlo + qc) * CHUNK
                Tq = s3p.tile([CHUNK, 512, 2], F32, tag=f"T{qc}")
                Tt.append(Tq)
                for ri in range(2):
                    ps = psp.tile([CHUNK, 512], F32, tag="s3t")
                    k = 0
                    for C in (0, 1):
                        for comp, gnm in ((0, "A"), (1, "B")):
                            nc.tensor.matmul(
                                ps[:],
                                mmcast(fkt[C][:, qs:qs + CHUNK, comp]),
                                mmcast(grt[(gnm, C)][:, :, ri]),
                                start=(k == 0), stop=(k == 3))
                            k += 1
                    nc.any.tensor_copy(Tq[:, :, ri], ps[:])
            for bc in range(4):
                ps = psp2.tile([CHUNK, 512], F32, tag="s3o")
                k = 0
                for qc in range(nq):
                    for comp, dram in ((0, gcA), (1, gcB)):
                        gct = s3p.tile([CHUNK, CHUNK], F32, tag="gc")
                        nc.sync.dma_start(
                            out=gct[:],
                            in_=dram[qc * CHUNK:(qc + 1) * CHUNK, bc * CHUNK:(bc + 1) * CHUNK])
                        nc.tensor.matmul(
                            ps[:], mmcast(gct[:]), mmcast(Tt[qc][:, :, comp]),
                            start=(k == 0), stop=(k == 2 * nq - 1))
                        k += 1
                ot = s3p.tile([CHUNK, 512], F32, tag="ot")
                nc.any.tensor_copy(ot[:], ps[:])
                nc.sync.dma_start(out=out[bc * CHUNK:(bc + 1) * CHUNK], in_=ot[:])
            phase3.__exit__(None, None, None)
    nc.compile()
    return nc


def core_inputs(S, tabs, sgm):
    """in_maps for the 8 cores of this class. Order: (b, r, f)."""
    ins = []
    for b in range(2):
        for r in S.bands:
            for f in (0, 1):
                t = tabs[(r, f)]
                im = {}
                im['sgmTa'] = np.ascontiguousarray(
                    sgm[b, 0][t['viewA']].transpose(0, 2, 1).astype(np.float32))
                im['sgmTb'] = np.ascontiguousarray(
                    sgm[b, 0][t['viewB']].transpose(0, 2, 1).astype(np.float32))
                for nm in ('cmat', 'wt', 'sc1', 'sc2', 'sc3', 'grA', 'grB', 'gcA', 'gcB'):
                    im[nm] = t[nm]
                ins.append(im)
    return ins


def combine_outputs(classes_results):
    """classes_results: list of (S, results8) in class order A,B -> rec [2,1,512,512]."""
    rec = np.zeros((2, 512, 512), np.float32)
    for S, res in classes_results:
        i = 0
        for b in range(2):
            for r in S.bands:
                for f in (0, 1):
                    o = res[i]['out']; i += 1
                    rec[b] += o.T if f == 0 else o
    return rec.reshape(2, 1, 512, 512)


_CACHE = {}


def _get_programs():
    if 'progs' not in _CACHE:
        classes = build_all()
        progs = []
        for S, tabs in classes:
            nc = build_program(S)
            progs.append((S, tabs, nc))
        _CACHE['progs'] = progs
    return _CACHE['progs']


def kernel(sgm):
    sgm = np.asarray(sgm, dtype=np.float32)
    assert sgm.shape == (2, 1, 512, 512)
    progs = _get_programs()
    results = []
    for S, tabs, nc in progs:
        ins = core_inputs(S, tabs, sgm)
        res = run_bass_kernel_spmd(nc, ins, core_ids=list(range(8)))
        results.append((S, res.results))
    return combine_outputs(results).astype(np.float32)



# revision 2
# speedup vs baseline: 1821.8885x; 1821.8885x over previous
"""Trainium2 Bass kernel for nn_DFTParallelRecon: polar-grid DFT CT reconstruction.

Self-contained: builds static geometry tables on host, compiles two SPMD Bass
programs (8 NeuronCores each; core = (BC, band, frame)), runs them sequentially,
and combines partial outputs on host.
"""
import sys
sys.path.insert(0, '/opt/trn_rl_repo')
import numpy as np
import concourse.bass as bass
import concourse.mybir as mybir
from concourse import bacc
from concourse.tile import TileContext
from concourse.masks import make_identity
from concourse.bass_utils import run_bass_kernel_spmd

"""Final tables: core = (BC b, band r, frame f). Two classes: bands {0,3}, {1,2}.
Each class -> one SPMD program over 8 cores (b2 x r2 x f2).
"""
import numpy as np

FM = 2048; V = 512; FN = 2048; M = 512; N_DET = 512; A_DET = 1.0; PIX = 0.5
TWO_PI = 2.0 * np.pi
C0 = 512; NB = 1024; BAND = 256
CHUNK = 128; NCH = 2
QSEG = 32; NG = CHUNK // QSEG
JSEG = 256; NJSEG = NB // JSEG
PIECE_CX = 255
DST_CAP16 = 2046


def polar_pix():
    """All nonzero pixels with reference-exact f32 bilinear data."""
    v = ((np.arange(FM, dtype=np.float32) - np.float32((FM - 1) / 2))
         * np.float32(1.0 / (FM * PIX))).astype(np.float32)
    ky, kx = np.meshgrid(v, v, indexing='xy')
    mk = np.abs(kx + 1j * ky).astype(np.float32)
    th = np.arctan2(ky, kx).astype(np.float32)
    theta = ((th + np.float32(TWO_PI)) / np.float32(TWO_PI) * np.float32(V)).astype(np.float32)
    k = (mk * np.float32(FN * A_DET) + np.float32((FN - 1) / 2)).astype(np.float32)
    theta_norm = ((theta - np.float32(V)) / np.float32(V)).astype(np.float32)
    k_norm = ((k - np.float32((FN - 1) / 2)) / np.float32(FN // 2)).astype(np.float32)
    ix = ((k_norm + np.float32(1.0)) * np.float32(0.5) * np.float32(FN - 1)).astype(np.float32)
    iy = ((theta_norm + np.float32(1.0)) * np.float32(0.5) * np.float32(2 * V - 1)).astype(np.float32)
    x0 = np.floor(ix).astype(np.int64); y0 = np.floor(iy).astype(np.int64)
    wx1 = (ix - x0).astype(np.float32); wy1 = (iy - y0).astype(np.float32)
    I, J = np.meshgrid(np.arange(FM), np.arange(FM), indexing='ij')
    c = (FM - 1) / 2.0
    u = I - c; w = J - c
    inA = np.abs(u) >= np.abs(w)
    v0 = (x0 >= 0) & (x0 <= FN - 1)
    v1 = (x0 + 1 >= 0) & (x0 + 1 <= FN - 1)
    sel = v0 | v1
    ii, jj = np.nonzero(sel)
    P = {}
    P['i'] = ii; P['j'] = jj
    P['frame'] = (~inA[ii, jj]).astype(np.int8)
    P['Y'] = y0[ii, jj]; P['x0'] = x0[ii, jj]
    _wx1 = wx1[ii, jj]; _wy1 = wy1[ii, jj]
    f0 = v0[ii, jj].astype(np.float32); f1 = v1[ii, jj].astype(np.float32)
    P['w0a'] = (1 - _wx1) * (1 - _wy1) * f0
    P['w1a'] = _wx1 * (1 - _wy1) * f1
    P['w0b'] = (1 - _wx1) * _wy1 * f0
    P['w1b'] = _wx1 * _wy1 * f1
    return P


def core_pix(P, r, f, mirror):
    """Pixel arrays for core (band r, frame f): local coords + chunk/slot/rank.
    mirror=True relabels fi -> BAND-1-fi, fj -> NB-1-fj so that the mirror band's
    table ranges align with its partner's; G tables absorb the flip."""
    frow = np.where(P['frame'] == 0, P['i'], P['j'])
    fcol = np.where(P['frame'] == 0, P['j'], P['i'])
    sel = (P['frame'] == f) & (frow >= C0 + r * BAND) & (frow < C0 + (r + 1) * BAND)
    d = {}
    for k in ('Y', 'x0', 'w0a', 'w1a', 'w0b', 'w1b'):
        d[k] = P[k][sel]
    d['fi'] = (frow[sel] - C0 - r * BAND).astype(np.int64)
    d['fj'] = (fcol[sel] - C0).astype(np.int64)
    d['mirror'] = mirror
    if mirror:
        d['fi'] = (BAND - 1) - d['fi']
        d['fj'] = (NB - 1) - d['fj']
    n = len(d['Y'])
    Ys = np.unique(d['Y'])
    assert len(Ys) <= NCH * CHUNK, f"{len(Ys)} wedges"
    ypos = {y: i for i, y in enumerate(Ys)}
    g = np.array([ypos[y] for y in d['Y']], np.int64)
    d['ch'] = g // CHUNK
    d['part'] = g % CHUNK
    d['Ylist'] = np.concatenate([Ys, np.full(NCH * CHUNK - len(Ys), Ys[-1])]).reshape(NCH, CHUNK)
    d['n'] = n
    d['q'] = d['fi'] % CHUNK
    d['C'] = d['fi'] // CHUNK
    return d


def runlen(key_sorted):
    n = len(key_sorted)
    same = np.concatenate([[False], key_sorted[1:] == key_sorted[:-1]])
    run = np.zeros(n, np.int64)
    for i in range(1, n):
        run[i] = run[i - 1] + 1 if same[i] else 0
    return run


def assign_kc(d, xlo):
    """k index within (ch, part, s) cell, and m within (ch, part, C, q)."""
    s = d['x0'] - xlo
    order = np.lexsort((d['fj'], d['fi'], s, d['part'], d['ch']))
    cell = (d['ch'][order] * CHUNK + d['part'][order]) * 4096 + s[order]
    kk = np.zeros(d['n'], np.int64); kk[order] = runlen(cell)
    order2 = np.lexsort((d['fj'], d['q'], d['C'], d['part'], d['ch']))
    cell2 = ((d['ch'][order2] * CHUNK + d['part'][order2]) * 2 + d['C'][order2]) * CHUNK + d['q'][order2]
    mm = np.zeros(d['n'], np.int64); mm[order2] = runlen(cell2)
    d['s'] = s; d['k'] = kk; d['m'] = mm


class ClassMeta:
    pass


def build_class(P, bands):
    """Build meta + per-core tables for one class. Cores: (b, r, f) for r in bands."""
    cores = []          # pixel dicts, one per (r, f); b doesn't affect tables
    for r in bands:
        for f in (0, 1):
            cores.append((r, f, core_pix(P, r, f, mirror=(r >= 2))))
    S = ClassMeta()
    S.bands = bands
    S.xlo = min(int(d['x0'].min()) for _, _, d in cores)
    xext = max(int(d['x0'].max()) for _, _, d in cores) - S.xlo + 1
    for _, _, d in cores:
        assign_kc(d, S.xlo)
    # pieces: smallest NP with per-window rank count <= PIECE_CX
    for NP in range(4, 24):
        pw = int(np.ceil(xext / NP))
        ok = True
        for _, _, d in cores:
            cnt = np.zeros((NCH, CHUNK, NP), np.int64)
            np.add.at(cnt, (d['ch'], d['part'], np.minimum(d['s'] // pw, NP - 1)), 1)
            if cnt.max() > PIECE_CX:
                ok = False; break
        if ok:
            break
    assert ok
    S.NP = NP; S.pw = pw; S.xpad = pw * NP
    # per (ch, piece) K maxed over cores
    Kmap = np.ones((NCH, NP), np.int64)
    for _, _, d in cores:
        cnt = np.zeros((NCH, CHUNK, S.xpad), np.int64)
        np.add.at(cnt, (d['ch'], d['part'], d['s']), 1)
        for w in range(NCH):
            for pc in range(NP):
                Kmap[w, pc] = max(Kmap[w, pc], int(cnt[w, :, pc * pw:(pc + 1) * pw].max()))
    S.pieces = [[(pc * pw, (pc + 1) * pw, int(Kmap[w, pc])) for pc in range(NP)]
                for w in range(NCH)]
    S.wt_off = []; S.sc1_off = []
    for w in range(NCH):
        wo = []; so = []; cw = 0; cs = 0
        for (a, b, K) in S.pieces[w]:
            wo.append(cw); cw += (b - a) * 4 * K
            so.append(cs); cs += (b - a) * K * 4
        S.wt_off.append(wo); S.sc1_off.append(so)
    S.wt_w = max(sum((b - a) * 4 * K for (a, b, K) in S.pieces[w]) for w in range(NCH))
    S.ncpx = PIECE_CX * NP
    # compact positions (per core)
    for _, _, d in cores:
        piece = d['s'] // pw
        order = np.lexsort((d['k'], d['s'], d['part'], piece, d['ch']))
        key = (d['ch'][order] * NP + piece[order]) * CHUNK + d['part'][order]
        run = runlen(key)
        cpos = np.zeros(d['n'], np.int64); cpos[order] = run
        assert cpos.max() < PIECE_CX
        d['piece'] = piece
        d['cpos'] = cpos + piece * PIECE_CX
    # outbox tiles per (w, C)
    S.Tn = {}
    for w in range(NCH):
        for C in (0, 1):
            t = 1
            for _, _, d in cores:
                msk = (d['ch'] == w) & (d['C'] == C)
                if msk.any():
                    t = max(t, int(d['m'][msk].max()) + 1)
            S.Tn[(w, C)] = t
    S.inbox_off = {}; S.inbox_w = {}
    for C in (0, 1):
        off = []; cur = 0
        for w in range(NCH):
            off.append(cur); cur += CHUNK * S.Tn[(w, C)] * 2
        S.inbox_off[C] = off; S.inbox_w[C] = cur
    # sc2 ranges per (w, C, g) maxed over cores
    S.sc2_rng = {}
    for _, _, d in cores:
        for w in range(NCH):
            for C in (0, 1):
                for g in range(NG):
                    msk = (d['ch'] == w) & (d['C'] == C) & (d['q'] // QSEG == g)
                    if not msk.any():
                        continue
                    lo, hi = int(d['cpos'][msk].min()), int(d['cpos'][msk].max()) + 1
                    key = (w, C, g)
                    if key in S.sc2_rng:
                        lo = min(lo, S.sc2_rng[key][0]); hi = max(hi, S.sc2_rng[key][1])
                    S.sc2_rng[key] = (lo, hi)
    S.sc2_off = {}; cur = 0
    for w in range(NCH):
        for C in (0, 1):
            for g in range(NG):
                lo, hi = S.sc2_rng.get((w, C, g), (0, 1))
                S.sc2_rng[(w, C, g)] = (lo, hi)
                S.sc2_off[(w, C, g)] = cur
                cur += (hi - lo) * 4
    S.sc2_w = cur
    # sc3 ranges per (C, jg)
    S.sc3_rng = {}
    for _, _, d in cores:
        for C in (0, 1):
            offs = S.inbox_off[C]
            ipos = offs_ipos(d, S, C)
            for jg in range(NJSEG):
                msk = (d['C'] == C) & (d['fj'] // JSEG == jg)
                if not msk.any():
                    continue
                lo, hi = int(ipos[msk].min()), int(ipos[msk].max()) + 2
                key = (C, jg)
                if key in S.sc3_rng:
                    lo = min(lo, S.sc3_rng[key][0]); hi = max(hi, S.sc3_rng[key][1])
                S.sc3_rng[key] = (lo, hi)
    S.sc3_off = {}; cur = 0
    for C in (0, 1):
        for jg in range(NJSEG):
            lo, hi = S.sc3_rng.get((C, jg), (0, 1))
            S.sc3_rng[(C, jg)] = (lo, hi)
            S.sc3_off[(C, jg)] = cur
            cur += (hi - lo) * 2
    S.sc3_w = cur
    # stage-3 q-chunk trim
    qlo = 8; qhi = 0
    for _, _, d in cores:
        qlo = min(qlo, int(d['fj'].min()) // CHUNK)
        qhi = max(qhi, int(d['fj'].max()) // CHUNK + 1)
    S.qlo, S.qhi = qlo, qhi
    # emit per-core tables
    tabs = {}
    for r, f, d in cores:
        tabs[(r, f)] = emit_core(d, S, r)
    return S, tabs


def offs_ipos(d, S, C):
    offs = S.inbox_off[C]
    ipos = np.zeros(d['n'], np.int64)
    for w in range(NCH):
        mw = d['ch'] == w
        ipos[mw] = offs[w] + (d['part'][mw] * S.Tn[(w, C)] + d['m'][mw]) * 2
    return ipos


def stage1_C():
    dx = A_DET; dk = 1.0 / (FN * dx)
    x0 = -(N_DET - 1) / 2 * dx; k0 = -(FN - 1) / 2 * dk
    m32 = np.arange(N_DET, dtype=np.float32)
    n32 = np.arange(FN, dtype=np.float32)
    ph_pre = (np.float32(TWO_PI * (k0 * dx)) * m32).astype(np.float32)
    pre = np.exp(-1j * ph_pre.astype(np.float64))
    inner = (np.float32(dk) * n32 + np.float32(k0)).astype(np.float32)
    ph_post = (np.float32(TWO_PI * x0) * inner).astype(np.float32)
    post = dx * np.exp(-1j * ph_post.astype(np.float64))
    mm = np.arange(N_DET, dtype=np.float64)
    nn = np.arange(FN, dtype=np.float64)
    Wm = np.exp(-1j * TWO_PI * np.outer(mm, nn) / FN)
    return ((pre[:, None] * Wm) * post[None, :]).astype(np.complex64)


def stage3_G():
    dx = PIX; dk = 1.0 / (FM * dx)
    x0 = -(FM - 1) / 2 * dx; k0 = -(FM - 1) / 2 * dk
    ar32 = np.arange(FM, dtype=np.float32)
    ph_pre = (np.float32(TWO_PI * (x0 * dk)) * ar32).astype(np.float32)
    pre = np.exp(1j * ph_pre.astype(np.float64))
    inner = (np.float32(dx) * ar32 + np.float32(x0)).astype(np.float32)
    ph_post = (np.float32(TWO_PI * k0) * inner).astype(np.float32)
    post = np.exp(1j * ph_post.astype(np.float64))
    lo = (FM - M) // 2
    p = np.arange(lo, lo + M)
    mm = np.arange(C0, C0 + NB)
    G = (dk * post[p])[:, None] * np.exp(1j * TWO_PI * np.outer(p, mm) / FM) * pre[mm][None, :]
    return G.astype(np.complex64)


_CM = None; _G = None


def emit_core(d, S, r):
    global _CM, _G
    if _CM is None:
        _CM = stage1_C(); _G = stage3_G()
    mirror = d['mirror']
    t = {}
    t['viewA'] = (d['Ylist'] % V).astype(np.int32)
    t['viewB'] = ((d['Ylist'] + 1) % V).astype(np.int32)
    n = d['n']
    ch, part, s, k, piece, cpos = d['ch'], d['part'], d['s'], d['k'], d['piece'], d['cpos']
    wt = np.zeros((NCH, CHUNK, S.wt_w), np.float16)
    sc1 = np.full((NCH, CHUNK, S.wt_w), -1, np.int16)
    for w in range(NCH):
        for pc, (a, b, K) in enumerate(S.pieces[w]):
            msk = (ch == w) & (piece == pc)
            if not msk.any():
                continue
            off = S.wt_off[w][pc]
            for di, nm in enumerate(('w0a', 'w1a', 'w0b', 'w1b')):
                pos = off + ((s[msk] - a) * 4 + di) * K + k[msk]
                wt[w, part[msk], pos] = d[nm][msk].astype(np.float16)
            soff = S.sc1_off[w][pc]
            cpl = cpos[msk] - pc * PIECE_CX
            for h in range(4):
                src = soff + ((s[msk] - a) * K + k[msk]) * 4 + h
                sc1[w, part[msk], src] = (cpl * 4 + h).astype(np.int16)
    t['wt'] = wt; t['sc1'] = sc1
    sc2 = np.full((CHUNK, S.sc2_w), -1, np.int16)
    for w in range(NCH):
        for C in (0, 1):
            Tn = S.Tn[(w, C)]
            for g in range(NG):
                lo, hi = S.sc2_rng[(w, C, g)]
                off = S.sc2_off[(w, C, g)]
                msk = (ch == w) & (d['C'] == C) & (d['q'] // QSEG == g)
                if not msk.any():
                    continue
                ql = d['q'][msk] - g * QSEG
                dstp = (ql * Tn + d['m'][msk]) * 2
                srcp = (cpos[msk] - lo) * 4
                assert int(dstp.max()) * 2 + 3 < QSEG * Tn * 4 <= DST_CAP16
                for h in range(4):
                    sc2[part[msk], off + srcp + h] = (dstp * 2 + h).astype(np.int16)
    t['sc2'] = sc2
    sc3 = np.full((CHUNK, S.sc3_w), -1, np.int16)
    for C in (0, 1):
        ipos = offs_ipos(d, S, C)
        for jg in range(NJSEG):
            lo, hi = S.sc3_rng[(C, jg)]
            off = S.sc3_off[(C, jg)]
            msk = (d['C'] == C) & (d['fj'] // JSEG == jg)
            if not msk.any():
                continue
            jl = d['fj'][msk] - jg * JSEG
            for h in range(4):
                ri = h // 2; half = h % 2
                sc3[d['q'][msk], off + (ipos[msk] - lo + ri) * 2 + half] = (
                    (jl * 2 + ri) * 2 + half).astype(np.int16)
    t['sc3'] = sc3
    # stage-1 C window [512, xpad+1, 2]
    xwin = np.zeros((N_DET, S.xpad + 1), np.complex64)
    hi = min(S.xlo + S.xpad + 1, FN)
    xwin[:, :hi - S.xlo] = _CM[:, S.xlo:hi]
    t['cmat'] = np.ascontiguousarray(np.stack([xwin.real, xwin.imag], -1).astype(np.float32))
    fi_loc = np.arange(BAND)
    if mirror:
        fi_loc = (BAND - 1) - fi_loc
    rows = r * BAND + fi_loc
    Gr = _G[:, rows].T
    t['grA'] = np.ascontiguousarray(np.stack([Gr.real, Gr.imag], -1).astype(np.float32))
    t['grB'] = np.ascontiguousarray(np.stack([-Gr.imag, Gr.real], -1).astype(np.float32))
    fj_loc = np.arange(S.qlo * CHUNK, S.qhi * CHUNK)
    if mirror:
        fj_loc = (NB - 1) - fj_loc
    Gq = _G.T[fj_loc]
    t['gcA'] = np.ascontiguousarray(Gq.real.astype(np.float32))
    t['gcB'] = np.ascontiguousarray((-Gq.imag).astype(np.float32))
    return t


def build_all():
    P = polar_pix()
    SA, tabsA = build_class(P, (0, 3))
    SB, tabsB = build_class(P, (1, 2))
    return (SA, tabsA), (SB, tabsB)


# ---------------- mock (device semantics) ----------------

def local_scatter_np(dst16, data16, idx16):
    dst16[:] = 0
    for prt in range(dst16.shape[0]):
        ii = idx16[prt]
        msk = ii >= 0
        dst16[prt, ii[msk].astype(np.int64)] = data16[prt, np.nonzero(msk)[0]]


def mock_core(sgm_b, t, S):
    Cm = t['cmat'][..., 0] + 1j * t['cmat'][..., 1]
    inbox = {C: np.zeros((CHUNK, S.inbox_w[C]), np.float32) for C in (0, 1)}
    compacts = []
    for w in range(NCH):
        Pa = sgm_b[t['viewA'][w]].astype(np.complex64) @ Cm
        Pb = sgm_b[t['viewB'][w]].astype(np.complex64) @ Cm
        Pa = np.stack([Pa.real, Pa.imag], -1).astype(np.float32)
        Pb = np.stack([Pb.real, Pb.imag], -1).astype(np.float32)
        compact = np.zeros((CHUNK, S.ncpx * 4), np.int16)
        for pc, (a, b, K) in enumerate(S.pieces[w]):
            pw = b - a
            wtp = t['wt'][w][:, S.wt_off[w][pc]:S.wt_off[w][pc] + pw * 4 * K]
            wtp = wtp.reshape(CHUNK, pw, 4, K).astype(np.float32)
            Vp = (wtp[:, :, 0, :, None] * Pa[:, a:b, None, :]
                  + wtp[:, :, 1, :, None] * Pa[:, a + 1:b + 1, None, :]
                  + wtp[:, :, 2, :, None] * Pb[:, a:b, None, :]
                  + wtp[:, :, 3, :, None] * Pb[:, a + 1:b + 1, None, :]).astype(np.float32)
            V16 = np.ascontiguousarray(Vp).view(np.int16).reshape(CHUNK, -1)
            idx = t['sc1'][w][:, S.sc1_off[w][pc]:S.sc1_off[w][pc] + pw * K * 4]
            local_scatter_np(compact[:, pc * PIECE_CX * 4:(pc + 1) * PIECE_CX * 4], V16, idx)
        compacts.append(compact)
    for C in (0, 1):
        for w in range(NCH):
            Tn = S.Tn[(w, C)]
            outbox = np.zeros((CHUNK, CHUNK, Tn, 2), np.float32)
            ob16 = outbox.view(np.int16).reshape(CHUNK, -1)
            for g in range(NG):
                lo, hi = S.sc2_rng[(w, C, g)]
                off = S.sc2_off[(w, C, g)]
                idx = t['sc2'][:, off:off + (hi - lo) * 4]
                local_scatter_np(ob16[:, g * QSEG * Tn * 4:(g + 1) * QSEG * Tn * 4],
                                 compacts[w][:, lo * 4:hi * 4], idx)
            ib = inbox[C]; ioff = S.inbox_off[C][w]
            for m in range(Tn):
                for ri in range(2):
                    ib[:, ioff + m * 2 + ri: ioff + CHUNK * Tn * 2: Tn * 2] = \
                        outbox[:, :, m, ri].T
    fkbuf = {}
    for C in (0, 1):
        fk = np.zeros((CHUNK, NB, 2), np.float32)
        fk16 = fk.view(np.int16).reshape(CHUNK, -1)
        ib16 = inbox[C].view(np.int16).reshape(CHUNK, -1)
        for jg in range(NJSEG):
            lo, hi = S.sc3_rng[(C, jg)]
            off = S.sc3_off[(C, jg)]
            idx = t['sc3'][:, off:off + (hi - lo) * 2]
            local_scatter_np(fk16[:, jg * JSEG * 4:(jg + 1) * JSEG * 4],
                             ib16[:, lo * 2:hi * 2], idx)
        fkbuf[C] = fk
    # stage 3 (single frame)
    fkc = np.concatenate([fkbuf[0], fkbuf[1]], axis=0)      # [256p, NB, 2]
    nq = (S.qhi - S.qlo) * CHUNK
    fkq = fkc[:, S.qlo * CHUNK:S.qhi * CHUNK]
    Tq = np.zeros((nq, 512, 2), np.float32)
    for ri in range(2):
        Tq[..., ri] = (np.einsum('pq,pa->qa', fkq[..., 0], t['grA'][..., ri])
                       + np.einsum('pq,pa->qa', fkq[..., 1], t['grB'][..., ri]))
    U = t['gcA'].T @ Tq[..., 0] + t['gcB'].T @ Tq[..., 1]
    return U.astype(np.float32)   # out[b-idx, a-idx] = U_f[a, b]


def full_mock(sgm, classes):
    rec = np.zeros((2, M, M), np.float32)
    for (S, tabs) in classes:
        for (r, f), t in tabs.items():
            for b in range(2):
                out = mock_core(sgm[b, 0], t, S)
                rec[b] += out.T if f == 0 else out
    return rec.reshape(2, 1, M, M)






F32 = mybir.dt.float32
F32R = mybir.dt.float32r
F16 = mybir.dt.float16
I16 = mybir.dt.int16


def build_program(S, use_f32r=False, repeat=1):
    nc = bacc.Bacc("TRN2", target_bir_lowering=False)
    xw = S.xpad + 1
    nq = S.qhi - S.qlo

    def mmcast(ap):
        return ap.bitcast(F32R) if use_f32r else ap

    sgmTa = nc.dram_tensor("sgmTa", [NCH, N_DET, CHUNK], F32, kind="ExternalInput")
    sgmTb = nc.dram_tensor("sgmTb", [NCH, N_DET, CHUNK], F32, kind="ExternalInput")
    cmat = nc.dram_tensor("cmat", [N_DET, xw, 2], F32, kind="ExternalInput")
    wt = nc.dram_tensor("wt", [NCH, CHUNK, S.wt_w], F16, kind="ExternalInput")
    sc1 = nc.dram_tensor("sc1", [NCH, CHUNK, S.wt_w], I16, kind="ExternalInput")
    sc2 = nc.dram_tensor("sc2", [CHUNK, S.sc2_w], I16, kind="ExternalInput")
    sc3 = nc.dram_tensor("sc3", [CHUNK, S.sc3_w], I16, kind="ExternalInput")
    grA = nc.dram_tensor("grA", [2 * CHUNK, 512, 2], F32, kind="ExternalInput")
    grB = nc.dram_tensor("grB", [2 * CHUNK, 512, 2], F32, kind="ExternalInput")
    gcA = nc.dram_tensor("gcA", [nq * CHUNK, 512], F32, kind="ExternalInput")
    gcB = nc.dram_tensor("gcB", [nq * CHUNK, 512], F32, kind="ExternalInput")
    out = nc.dram_tensor("out", [512, 512], F32, kind="ExternalOutput")

    with TileContext(nc) as tc:
      import contextlib
      loop_cm = tc.For_i(0, repeat) if repeat > 1 else contextlib.nullcontext()
      with loop_cm:
        with (
            tc.tile_pool(name="const", bufs=1) as constp,
            tc.tile_pool(name="compact", bufs=1) as cp,
            tc.tile_pool(name="fkpool", bufs=1) as fkp,
            tc.tile_pool(name="psum", bufs=2, space="PSUM") as psp,
            tc.tile_pool(name="psum2", bufs=2, space="PSUM") as psp2,
        ):
            ident = constp.tile([CHUNK, CHUNK], F32)
            make_identity(nc, ident[:])
            cmt = []
            for kc in range(4):
                t = constp.tile([CHUNK, xw, 2], F32, tag=f"cm{kc}")
                nc.sync.dma_start(out=t[:], in_=cmat[kc * CHUNK:(kc + 1) * CHUNK])
                cmt.append(t)
            grt = {}
            for nm, dram in (("A", grA), ("B", grB)):
                for kc in range(2):
                    t = constp.tile([CHUNK, 512, 2], F32, tag=f"gr{nm}{kc}")
                    nc.sync.dma_start(out=t[:], in_=dram[kc * CHUNK:(kc + 1) * CHUNK])
                    grt[(nm, kc)] = t

            compacts = []
            for w in range(NCH):
                cpt = cp.tile([CHUNK, S.ncpx * 4], I16, tag=f"cpt{w}", name=f"cpt{w}")
                compacts.append(cpt)

            phase1 = tc.tile_pool(name="phase1", bufs=2)
            pp = vp = phase1.__enter__()
            for w in range(NCH):
                sg = {}
                for nm, dram in (("a", sgmTa), ("b", sgmTb)):
                    for kc in range(4):
                        t = pp.tile([CHUNK, CHUNK], F32, tag=f"sg{nm}{kc}")
                        nc.sync.dma_start(out=t[:], in_=dram[w, kc * CHUNK:(kc + 1) * CHUNK])
                        sg[(nm, kc)] = t
                Pab = {}
                for nm in ("a", "b"):
                    P = pp.tile([CHUNK, xw, 2], F32, tag=f"P{nm}")
                    Pab[nm] = P
                    for ri in range(2):
                        xs = 0
                        while xs < xw:
                            pl = min(512, xw - xs)
                            ps = psp.tile([CHUNK, 512], F32, tag="s1")
                            for kc in range(4):
                                nc.tensor.matmul(
                                    ps[:, :pl],
                                    mmcast(sg[(nm, kc)][:]),
                                    mmcast(cmt[kc][:, xs:xs + pl, ri]),
                                    start=(kc == 0), stop=(kc == 3))
                            nc.any.tensor_copy(P[:, xs:xs + pl, ri], ps[:, :pl])
                            xs += pl
                # V pieces + sc1
                for pc, (a, b, K) in enumerate(S.pieces[w]):
                    pw = b - a
                    wtile = vp.tile([CHUNK, pw, 4, K], F16, tag="wt")
                    nc.sync.dma_start(
                        out=wtile[:],
                        in_=wt[w, :, S.wt_off[w][pc]:S.wt_off[w][pc] + pw * 4 * K])
                    idxt = vp.tile([CHUNK, pw * K * 4], I16, tag="sc1i")
                    nc.sync.dma_start(
                        out=idxt[:],
                        in_=sc1[w, :, S.sc1_off[w][pc]:S.sc1_off[w][pc] + pw * K * 4])
                    V = vp.tile([CHUNK, pw, K, 2], F32, tag="V")
                    t1 = vp.tile([CHUNK, pw, K, 2], F32, tag="t1")
                    shp = (CHUNK, pw, K, 2)
                    terms = [("a", 0, 0), ("a", 1, 1), ("b", 0, 2), ("b", 1, 3)]
                    first = True
                    for nm, dx, di in terms:
                        wap = wtile[:, :, di, :, None].to_broadcast(shp)
                        pap = Pab[nm][:, a + dx:b + dx, None, :].to_broadcast(shp)
                        dst = V if first else t1
                        nc.vector.tensor_tensor(out=dst[:], in0=wap, in1=pap,
                                                op=mybir.AluOpType.mult)
                        if not first:
                            nc.vector.tensor_tensor(out=V[:], in0=V[:], in1=t1[:],
                                                    op=mybir.AluOpType.add)
                        first = False
                    vflat = V[:].rearrange("p a k r -> p (a k r)").bitcast(I16)
                    nc.gpsimd.local_scatter(
                        compacts[w][:, pc * PIECE_CX * 4:(pc + 1) * PIECE_CX * 4],
                        vflat, idxt[:],
                        channels=CHUNK, num_elems=PIECE_CX * 4, num_idxs=pw * K * 4)

            phase1.__exit__(None, None, None)
            # routing + fk
            phase2 = tc.tile_pool(name="phase2", bufs=2)
            rp = phase2.__enter__()
            fkt = {}
            for C in (0, 1):
                inbox = rp.tile([CHUNK, S.inbox_w[C]], F32, tag=f"inbox{C}", bufs=1)
                for w in range(NCH):
                    Tn = S.Tn[(w, C)]
                    ob = rp.tile([CHUNK, CHUNK, Tn, 2], F32, tag="outbox")
                    for g in range(NG):
                        lo, hi = S.sc2_rng[(w, C, g)]
                        off = S.sc2_off[(w, C, g)]
                        idxt = rp.tile([CHUNK, (hi - lo) * 4], I16, tag="sc2i")
                        nc.sync.dma_start(out=idxt[:], in_=sc2[:, off:off + (hi - lo) * 4])
                        dst = ob[:, g * QSEG:(g + 1) * QSEG, :, :]
                        dst = dst.rearrange("p q t r -> p (q t r)").bitcast(I16)
                        nc.gpsimd.local_scatter(
                            dst, compacts[w][:, lo * 4:hi * 4], idxt[:],
                            channels=CHUNK, num_elems=QSEG * Tn * 4,
                            num_idxs=(hi - lo) * 4)
                    ioff = S.inbox_off[C][w]
                    isec = inbox[:, ioff:ioff + CHUNK * Tn * 2]
                    isec = isec.rearrange("p (l t r) -> p l t r", t=Tn, r=2)
                    for m in range(Tn):
                        for ri in range(2):
                            pt = psp.tile([CHUNK, CHUNK], F32, tag="tp")
                            nc.tensor.transpose(pt[:], ob[:, :, m, ri], ident[:])
                            nc.any.tensor_copy(isec[:, :, m, ri], pt[:])
                fk = fkp.tile([CHUNK, NB, 2], F32, tag=f"fk{C}")
                fkt[C] = fk
                ibflat = inbox[:].rearrange("p x -> p x").bitcast(I16)
                for jg in range(NJSEG):
                    lo, hi = S.sc3_rng[(C, jg)]
                    off = S.sc3_off[(C, jg)]
                    idxt = rp.tile([CHUNK, (hi - lo) * 2], I16, tag="sc3i")
                    nc.sync.dma_start(out=idxt[:], in_=sc3[:, off:off + (hi - lo) * 2])
                    dst = fk[:, jg * JSEG:(jg + 1) * JSEG, :]
                    dst = dst.rearrange("p j r -> p (j r)").bitcast(I16)
                    nc.gpsimd.local_scatter(
                        dst, ibflat[:, lo * 2:hi * 2], idxt[:],
                        channels=CHUNK, num_elems=JSEG * 4, num_idxs=(hi - lo) * 2)

            phase2.__exit__(None, None, None)
            # stage 3
            phase3 = tc.tile_pool(name="phase3", bufs=2)
            s3p = phase3.__enter__()
            Tt = []
            for qc in range(nq):
                qs = (S.qlo + qc) * CHUNK
                Tq = s3p.tile([CHUNK, 512, 2], F32, tag=f"T{qc}")
                Tt.append(Tq)
                for ri in range(2):
                    ps = psp.tile([CHUNK, 512], F32, tag="s3t")
                    k = 0
                    for C in (0, 1):
                        for comp, gnm in ((0, "A"), (1, "B")):
                            nc.tensor.matmul(
                                ps[:],
                                mmcast(fkt[C][:, qs:qs + CHUNK, comp]),
                                mmcast(grt[(gnm, C)][:, :, ri]),
                                start=(k == 0), stop=(k == 3))
                            k += 1
                    nc.any.tensor_copy(Tq[:, :, ri], ps[:])
            for bc in range(4):
                ps = psp2.tile([CHUNK, 512], F32, tag="s3o")
                k = 0
                for qc in range(nq):
                    for comp, dram in ((0, gcA), (1, gcB)):
                        gct = s3p.tile([CHUNK, CHUNK], F32, tag="gc")
                        nc.sync.dma_start(
                            out=gct[:],
                            in_=dram[qc * CHUNK:(qc + 1) * CHUNK, bc * CHUNK:(bc + 1) * CHUNK])
                        nc.tensor.matmul(
                            ps[:], mmcast(gct[:]), mmcast(Tt[qc][:, :, comp]),
                            start=(k == 0), stop=(k == 2 * nq - 1))
                        k += 1
                ot = s3p.tile([CHUNK, 512], F32, tag="ot")
                nc.any.tensor_copy(ot[:], ps[:])
                nc.sync.dma_start(out=out[bc * CHUNK:(bc + 1) * CHUNK], in_=ot[:])
            phase3.__exit__(None, None, None)
    nc.compile()
    return nc


def core_inputs(S, tabs, sgm):
    """in_maps for the 8 cores of this class. Order: (b, r, f)."""
    ins = []
    for b in range(2):
        for r in S.bands:
            for f in (0, 1):
                t = tabs[(r, f)]
                im = {}
                im['sgmTa'] = np.ascontiguousarray(
                    sgm[b, 0][t['viewA']].transpose(0, 2, 1).astype(np.float32))
                im['sgmTb'] = np.ascontiguousarray(
                    sgm[b, 0][t['viewB']].transpose(0, 2, 1).astype(np.float32))
                for nm in ('cmat', 'wt', 'sc1', 'sc2', 'sc3', 'grA', 'grB', 'gcA', 'gcB'):
                    im[nm] = t[nm]
                ins.append(im)
    return ins


def combine_outputs(classes_results):
    """classes_results: list of (S, results8) in class order A,B -> rec [2,1,512,512]."""
    rec = np.zeros((2, 512, 512), np.float32)
    for S, res in classes_results:
        i = 0
        for b in range(2):
            for r in S.bands:
                for f in (0, 1):
                    o = res[i]['out']; i += 1
                    rec[b] += o.T if f == 0 else o
    return rec.reshape(2, 1, 512, 512)


_CACHE = {}


def _get_programs():
    if 'progs' not in _CACHE:
        classes = build_all()
        progs = []
        for S, tabs in classes:
            nc = build_program(S)
            progs.append((S, tabs, nc))
        _CACHE['progs'] = progs
    return _CACHE['progs']


def kernel(sgm):
    sgm = np.asarray(sgm, dtype=np.float32)
    assert sgm.shape == (2, 1, 512, 512)
    progs = _get_programs()
    results = []
    for S, tabs, nc in progs:
        ins = core_inputs(S, tabs, sgm)
        res = run_bass_kernel_spmd(nc, ins, core_ids=list(range(8)))
        results.append((S, res.results))
    return combine_outputs(results).astype(np.float32)

